# revision 6
# baseline (speedup 1.0000x reference)
"""Trainium2 Bass kernel for nn_MoE_90297392431448.

MoE layer: B=2, T=2048, D=1024, H=4096, E=8 experts, top-K=2 routing.

Strategy (expert-parallel, routed):
  - Host: gating softmax + top-2 selection in fp64 (tiny: 4096x1024 @ 1024x8),
    renormalized gate weights; gather each expert's tokens.
  - Device (8 cores, SPMD, one expert per core): two-stage FFN
        hT = gelu(W1.T @ xT + b1)          [H, M]  (partitions = H-chunks)
        yT = (W2.T @ hT) * w[m]            [D, M]  (partitions = D-chunks)
    in bf16 matmuls with fp32 PSUM accumulation.
  - Host: transpose + scatter-add per-expert outputs (+ w*b2 rank-1 term).

Perf model (measured): matmul pitch = free_rows * 0.4167ns (1 row/cycle at
2.4GHz) + ~2.7ns issue overhead; no LDWEIGHTS penalty for free >= 256. The
PE clock is HAM-gated: 1.2GHz until ~3.4us of *continuous* activity, and any
multi-us idle gap re-throttles it. DMA: the sync-engine queue (q1) sustains
~155-227GB/s from the start; gpsimd/scalar queues crawl (~60GB/s) during the
first ~10us, so all ramp-critical transfers ride q1.

v3 changes vs the 270.5us baseline:
  - Stage 2 streams ht as the moving operand (m = free dim, output [D, M]
    per-block): cost is proportional to M rather than to 128-token tile
    count, and per-token gate weights apply via a vector tensor_mul against
    a host-replicated [P, M] weight image. Host un-transposes the result.
  - M padded to 16 (1104 here) instead of 128 (1152).
  - Warmup: 9x512 dummy matmuls + 24x128 fine-grained bridge so the PE stays
    continuously busy from ~7.9us until the first real chain's data lands
    (~14us) -> HAM never re-throttles (the baseline lost ~2.2us to a K=4/8
    window at 14-18.5us).
  - W2 streams ungated on the gpsimd queue from t=0 (it only reaches full
    rate mid-kernel anyway); W1 keeps the whole sync queue.
"""

import numpy as np
import ml_dtypes

B, T, D, H, E, K = 2, 2048, 1024, 4096, 8, 2
P = 128
KD = D // P    # 8  k-chunks of the D contraction
CH = H // P    # 32 chunks of H
DK = D // P    # 8  d-chunks of stage-2 output
N_TOK = B * T

_compiled_cache = {}


def _m_blocks(M):
    """m blocks (offset, size), multiples of 16, each <= 512 (PSUM bank) and
    >= ~256 (stream-bound). Block0 = 512: the W1 stream (8.4MB on one queue)
    is consumed at ~200GB/s during block0's stage 1, which q1 just sustains."""
    assert M % 16 == 0 and M <= 1536
    if M <= 512:
        return [(0, M)]
    R = M - 512
    if R <= 512:
        return [(0, 512), (512, R)]
    s2 = max(224, 16 * (R // 2 // 16))
    s1 = R - s2
    return [(0, 512), (512, s1), (512 + s1, s2)]


def _build(M):
    """Build + compile the per-expert FFN kernel for capacity M (mult of 16)."""
    import concourse.bass as bass
    import concourse.mybir as mybir
    import concourse.tile as tile
    from concourse import bacc

    bf16 = mybir.dt.bfloat16
    f32 = mybir.dt.float32

    nc = bacc.Bacc("TRN2", target_bir_lowering=False, debug=False, num_devices=E)

    m_blocks = _m_blocks(M)

    # packed (SBUF-layout) DRAM images -> large contiguous DMA rows
    xt_d = nc.dram_tensor("xt", [P, KD * M], bf16, kind="ExternalInput")
    w1_d = nc.dram_tensor("w1", [P, KD * H], bf16, kind="ExternalInput")
    # W2 packed d-chunk-major: free idx = dk*(CH*P) + c*P + d'
    w2_d = nc.dram_tensor("w2", [P, DK * CH * P], bf16, kind="ExternalInput")
    b1_d = nc.dram_tensor("b1s", [P, CH], f32, kind="ExternalInput")
    wr_d = nc.dram_tensor("wrep", [P, M], f32, kind="ExternalInput")
    y_d = nc.dram_tensor("y", [D, M], f32, kind="ExternalOutput")

    with tile.TileContext(nc) as tc:
        with (
            tc.tile_pool(name="weights", bufs=1) as wpool,
            tc.tile_pool(name="xin", bufs=1) as xpool,
            tc.tile_pool(name="hbuf", bufs=1) as hpool,
            tc.tile_pool(name="obuf", bufs=4) as opool,
            tc.tile_pool(name="ps1", bufs=3, space="PSUM") as psum1,
            tc.tile_pool(name="ps2", bufs=3, space="PSUM") as psum2,
            tc.tile_pool(name="warm", bufs=1, space="PSUM") as pswarm,
        ):
            # Warm up the PE clock with dummy matmuls while the input DMAs
            # stream in: 9x512-free cover the bulk (~3.8us cold), then
            # 24x128-free bridge at ~0.1us granularity to just before the
            # first real chain's data lands, so the PE never idles (an idle
            # gap would re-throttle the clock to 1.2GHz for ~3.4us).
            wz = xpool.tile([P, 512], bf16, tag="warmsrc")
            nc.gpsimd.memset(wz[:], 0.0)
            pw_t = pswarm.tile([P, 512], f32, tag="warmps")
            for _ in range(9):
                nc.tensor.matmul(pw_t[:], wz[:, :P], wz[:], start=True, stop=True)
            for _ in range(24):
                nc.tensor.matmul(pw_t[:, :P], wz[:, :P], wz[:, :P],
                                 start=True, stop=True)
            xt_sb = xpool.tile([P, KD * M], bf16)
            w1_sb = wpool.tile([P, KD * H], bf16)
            w2_sb = wpool.tile([P, DK * CH * P], bf16)
            CB = 4096  # 1 MiB blocks
            mo0, mb0 = m_blocks[0]
            # ramp-critical stream, all on the sync queue (q1): the first
            # stage-1 c-groups need xt block0 k=0..3 + W1 MiB0 first half;
            # their k=4..7 halves follow.
            h1 = KD // 2 * mb0
            nc.sync.dma_start(xt_sb[:, :h1], xt_d.ap()[:, :h1])
            nc.sync.dma_start(w1_sb[:, :CB // 2], w1_d.ap()[:, :CB // 2])
            nc.sync.dma_start(xt_sb[:, h1:KD * mb0], xt_d.ap()[:, h1:KD * mb0])
            nc.sync.dma_start(w1_sb[:, CB // 2:CB], w1_d.ap()[:, CB // 2:CB])
            for cb in range(1, KD * H // CB):
                nc.sync.dma_start(w1_sb[:, cb * CB:(cb + 1) * CB],
                                  w1_d.ap()[:, cb * CB:(cb + 1) * CB])
            # scalar queue (slow early, fine later): gelu table preload
            # first, then biases, gate-weight image, xt blocks 1-2
            wg_t = xpool.tile([P, 1], bf16, tag="warmgelu")
            nc.scalar.activation(wg_t[:], wz[:, :1],
                                 mybir.ActivationFunctionType.Gelu)
            b1_sb = wpool.tile([P, CH], f32)
            nc.scalar.dma_start(b1_sb[:], b1_d.ap())
            wr_sb = wpool.tile([P, M], f32)
            nc.scalar.dma_start(wr_sb[:], wr_d.ap())
            if M > mb0:
                nc.scalar.dma_start(xt_sb[:, KD * mb0:], xt_d.ap()[:, KD * mb0:])
            # W2 on the gpsimd queue from t=0: q0 crawls for ~10us then runs
            # ~186GB/s -> 8.4MB lands ~55us, well before stage 2 needs its
            # later d-chunks (it consumes d-chunk-major from ~70us).
            for cb in range(DK * CH * P // CB):
                nc.gpsimd.dma_start(w2_sb[:, cb * CB:(cb + 1) * CB],
                                    w2_d.ap()[:, cb * CB:(cb + 1) * CB])

            def s1_mm(ps, c, k, fo, mb, start, stop):
                w1base = (c // 4) * 4096 + (c % 4) * P
                nc.tensor.matmul(
                    ps[:, :mb],
                    w1_sb[:, w1base + k * 512: w1base + k * 512 + P],
                    xt_sb[:, fo + k * mb: fo + (k + 1) * mb],
                    start=start,
                    stop=stop,
                )

            for bi, (mo, mb) in enumerate(m_blocks):
                fo = KD * mo
                # stage 1: hT[n, m] = gelu(sum_k W1[k, n] * x[m, k] + b1[n])
                ht = hpool.tile([P, CH, 512], bf16, tag="ht")
                c_start = 0
                if bi == 0:
                    # first three c-groups in two half-K passes: their k=0..3
                    # matmuls need only the first xt/W1 halves, so they run
                    # while the k=4..7 halves are still streaming in
                    c_start = 3
                    pss = []
                    for c in range(c_start):
                        ps = psum1.tile([P, 512], f32, tag="ps1")
                        pss.append(ps)
                        for k in range(KD // 2):
                            s1_mm(ps, c, k, fo, mb, k == 0, False)
                    for c in range(c_start):
                        ps = pss[c]
                        for k in range(KD // 2, KD):
                            s1_mm(ps, c, k, fo, mb, False, k == KD - 1)
                        nc.scalar.activation(
                            ht[:, c, :mb], ps[:, :mb],
                            mybir.ActivationFunctionType.Gelu,
                            bias=b1_sb[:, c:c + 1],
                        )
                for c in range(c_start, CH):
                    ps = psum1.tile([P, 512], f32, tag="ps1")
                    for k in range(KD):
                        s1_mm(ps, c, k, fo, mb, k == 0, k == KD - 1)
                    nc.scalar.activation(
                        ht[:, c, :mb], ps[:, :mb],
                        mybir.ActivationFunctionType.Gelu,
                        bias=b1_sb[:, c:c + 1],
                    )
                # stage 2: yT[d, m] = w[m] * sum_h W2[h, d] * hT[h, m]
                # (d-chunk stationary, ht moving: cost ~ mb, no 128-tiling)
                last_block = (bi == len(m_blocks) - 1)
                for dk in range(DK):
                    last_dk = last_block and dk == DK - 1
                    ps2 = psum2.tile([P, 512], f32, tag="ps2")
                    for c in range(CH):
                        nc.tensor.matmul(
                            ps2[:, :mb],
                            w2_sb[:, dk * CH * P + c * P: dk * CH * P + (c + 1) * P],
                            ht[:, c, :mb],
                            start=(c == 0),
                            stop=(c == CH - 1),
                        )
                    ot = opool.tile([P, 512], f32, tag="ot")
                    if last_dk:
                        # final chain: evict in halves, store each on its own
                        # queue immediately so the last store overlaps the
                        # second evict instead of serializing after it
                        hw = mb // 2
                        for q, eng in ((0, nc.sync), (1, nc.scalar)):
                            nc.vector.tensor_mul(
                                ot[:, q * hw:(q + 1) * hw],
                                ps2[:, q * hw:(q + 1) * hw],
                                wr_sb[:, mo + q * hw: mo + (q + 1) * hw])
                            eng.dma_start(
                                y_d.ap()[dk * P:(dk + 1) * P,
                                         mo + q * hw: mo + (q + 1) * hw],
                                ot[:, q * hw:(q + 1) * hw])
                    else:
                        nc.vector.tensor_mul(ot[:, :mb], ps2[:, :mb],
                                             wr_sb[:, mo:mo + mb])
                        nc.sync.dma_start(
                            y_d.ap()[dk * P:(dk + 1) * P, mo:mo + mb],
                            ot[:, :mb])
    nc.compile()
    return nc


def _route(x2d, Wg, bg):
    """fp64 gating: returns (top2 indices [N,2], renormalized weights [N,2])."""
    logits = x2d.astype(np.float64) @ Wg.astype(np.float64) + bg.astype(np.float64)
    m = logits.max(-1, keepdims=True)
    e = np.exp(logits - m)
    gates = e / e.sum(-1, keepdims=True)
    top2 = np.argsort(-gates, axis=-1, kind="stable")[:, :K]
    g2 = np.take_along_axis(gates, top2, axis=-1)
    w2 = g2 / np.maximum(g2.sum(-1, keepdims=True), 1e-12)
    return top2, w2


def kernel(x, Wg, bg, W1, b1, W2, b2, _run_opts=None):
    from concourse.bass_utils import run_bass_kernel_spmd

    x = np.asarray(x)
    x2d = x.reshape(N_TOK, D)
    top2, wgt2 = _route(x2d, np.asarray(Wg), np.asarray(bg))

    # per-expert token lists
    pos = [np.where((top2 == e).any(-1))[0] for e in range(E)]
    pw = [
        (wgt2 * (top2 == e))[pos[e]].sum(-1).astype(np.float32)
        for e in range(E)
    ]
    max_n = max(len(p) for p in pos)
    M = max(P, -(-max_n // 16) * 16)

    if M not in _compiled_cache:
        _compiled_cache[M] = _build(M)
    nc = _compiled_cache[M]

    bf = ml_dtypes.bfloat16
    W1 = np.asarray(W1)
    W2 = np.asarray(W2)
    b1 = np.asarray(b1)
    b2 = np.asarray(b2)

    m_blocks = _m_blocks(M)

    in_maps = []
    for e in range(E):
        n_e = len(pos[e])
        xt = np.zeros((D, M), bf)
        xt[:, :n_e] = x2d[pos[e]].T.astype(bf)
        # pack xt -> [P, sum_b KD*mb] with per-block [k, m'] free layout
        xt3 = xt.reshape(KD, P, M)
        xtp = np.empty((P, KD * M), bf)
        for (mo, mb) in m_blocks:
            blk = xt3[:, :, mo:mo + mb]          # [KD, P, mb]
            xtp[:, KD * mo:KD * (mo + mb)] = (
                blk.transpose(1, 0, 2).reshape(P, KD * mb))
        # pack W1 [D, H] -> [P, KD*H]: free idx = cb*4096 + k*512 + h''
        w1p = (W1[e].astype(bf)
               .reshape(KD, P, H // 512, 512)      # [k, p, cb, h'']
               .transpose(1, 2, 0, 3)              # [p, cb, k, h'']
               .reshape(P, KD * H))
        # pack W2 [H, D] -> [P, DK*CH*P]: free idx = dk*CH*128 + c*128 + d'
        w2p = (W2[e].astype(bf)
               .reshape(CH, P, DK, P)              # [c, p, dk, d']
               .transpose(1, 2, 0, 3)              # [p, dk, c, d']
               .reshape(P, DK * CH * P))
        # gate weights replicated across partitions: [P, M]
        w_full = np.zeros((M,), np.float32)
        w_full[:n_e] = pw[e]
        wrep = np.broadcast_to(w_full, (P, M))
        in_maps.append({
            "xt": np.ascontiguousarray(xtp),
            "w1": np.ascontiguousarray(w1p),
            "w2": np.ascontiguousarray(w2p),
            "b1s": np.ascontiguousarray(b1[e].reshape(CH, P).T.astype(np.float32)),
            "wrep": np.ascontiguousarray(wrep),
        })

    try:
        res = run_bass_kernel_spmd(nc, in_maps, core_ids=list(range(E)),
                                   **(_run_opts or {}))
    except Exception:
        # transient device errors (e.g. NRT_EXEC_UNIT_UNRECOVERABLE) have
        # been observed on this fabric; one retry usually clears them
        res = run_bass_kernel_spmd(nc, in_maps, core_ids=list(range(E)),
                                   **(_run_opts or {}))

    out = np.zeros((N_TOK, D), np.float32)
    for e in range(E):
        n_e = len(pos[e])
        if n_e == 0:
            continue
        y = res.results[e]["y"][:, :n_e]          # [D, n_e]
        out[pos[e]] += y.T + pw[e][:, None] * b2[e][None, :].astype(np.float32)
    if _run_opts is not None:
        kernel._last_result = res
    return out.reshape(B, T, D)


if __name__ == "__main__":
    rng = np.random.default_rng(0)
    ins = {
        "x": rng.standard_normal((B, T, D), dtype=np.float32),
        "Wg": rng.standard_normal((D, E), dtype=np.float32) * 0.03,
        "bg": rng.standard_normal((E,), dtype=np.float32) * 0.03,
        "W1": rng.standard_normal((E, D, H), dtype=np.float32) * 0.03,
        "b1": rng.standard_normal((E, H), dtype=np.float32) * 0.03,
        "W2": rng.standard_normal((E, H, D), dtype=np.float32) * 0.015,
        "b2": rng.standard_normal((E, D), dtype=np.float32) * 0.015,
    }
    out = kernel(**ins)
    print("kernel out:", out.shape, out.dtype, float(np.abs(out).mean()))


# revision 9
# speedup vs baseline: 1.0310x; 1.0310x over previous
"""Trainium2 Bass kernel for nn_MoE_90297392431448.

MoE layer: B=2, T=2048, D=1024, H=4096, E=8 experts, top-K=2 routing.

Strategy (expert-parallel, routed):
  - Host: gating softmax + top-2 selection in fp64 (tiny: 4096x1024 @ 1024x8),
    renormalized gate weights; gather each expert's tokens.
  - Device (8 cores, SPMD, one expert per core): two-stage FFN
        hT = gelu(W1.T @ xT + b1)          [H, M]  (partitions = H-chunks)
        yT = (W2.T @ hT) * w[m]            [D, M]  (partitions = D-chunks)
    in bf16 matmuls with fp32 PSUM accumulation.
  - Host: transpose + scatter-add per-expert outputs (+ w*b2 rank-1 term).

Perf model (measured): matmul pitch = free_rows * 0.4167ns (1 row/cycle at
2.4GHz) + ~2.7ns issue overhead; no LDWEIGHTS penalty for free >= 256. The
PE clock is HAM-gated: 1.2GHz until ~3.4us of *continuous* activity, and any
multi-us idle gap re-throttles it. DMA: the sync-engine queue (q1) sustains
~155-227GB/s from the start; gpsimd/scalar queues crawl (~60GB/s) during the
first ~10us, so all ramp-critical transfers ride q1.

v3 changes vs the 270.5us baseline:
  - Stage 2 streams ht as the moving operand (m = free dim, output [D, M]
    per-block): cost is proportional to M rather than to 128-token tile
    count, and per-token gate weights apply via a vector tensor_mul against
    a host-replicated [P, M] weight image. Host un-transposes the result.
  - M padded to 16 (1104 here) instead of 128 (1152).
  - Warmup: 9x512 dummy matmuls + 24x128 fine-grained bridge so the PE stays
    continuously busy from ~7.9us until the first real chain's data lands
    (~14us) -> HAM never re-throttles (the baseline lost ~2.2us to a K=4/8
    window at 14-18.5us).
  - W2 streams ungated on the gpsimd queue from t=0 (it only reaches full
    rate mid-kernel anyway); W1 keeps the whole sync queue.
"""

import numpy as np
import ml_dtypes

B, T, D, H, E, K = 2, 2048, 1024, 4096, 8, 2
P = 128
KD = D // P    # 8  k-chunks of the D contraction
CH = H // P    # 32 chunks of H
DK = D // P    # 8  d-chunks of stage-2 output
N_TOK = B * T

_compiled_cache = {}


def _m_blocks(M):
    """m blocks (offset, size), multiples of 16, each <= 512 (PSUM bank) and
    >= ~256 (stream-bound). Block0 = 512: the W1 stream (8.4MB on one queue)
    is consumed at ~200GB/s during block0's stage 1, which q1 just sustains."""
    assert M % 16 == 0 and M <= 1536
    if M <= 512:
        return [(0, M)]
    R = M - 512
    if R <= 512:
        return [(0, 512), (512, R)]
    s2 = max(224, 16 * (R // 2 // 16))
    s1 = R - s2
    return [(0, 512), (512, s1), (512 + s1, s2)]


def _build(M):
    """Build + compile the per-expert FFN kernel for capacity M (mult of 16)."""
    import concourse.bass as bass
    import concourse.mybir as mybir
    import concourse.tile as tile
    from concourse import bacc

    bf16 = mybir.dt.bfloat16
    f32 = mybir.dt.float32

    nc = bacc.Bacc("TRN2", target_bir_lowering=False, debug=False, num_devices=E)

    m_blocks = _m_blocks(M)

    # packed (SBUF-layout) DRAM images -> large contiguous DMA rows
    xt_d = nc.dram_tensor("xt", [P, KD * M], bf16, kind="ExternalInput")
    w1_d = nc.dram_tensor("w1", [P, KD * H], bf16, kind="ExternalInput")
    # W2 packed d-chunk-major: free idx = dk*(CH*P) + c*P + d'
    w2_d = nc.dram_tensor("w2", [P, DK * CH * P], bf16, kind="ExternalInput")
    b1_d = nc.dram_tensor("b1s", [P, CH], f32, kind="ExternalInput")
    wr_d = nc.dram_tensor("wrep", [P, M], f32, kind="ExternalInput")
    y_d = nc.dram_tensor("y", [D, M], f32, kind="ExternalOutput")

    with tile.TileContext(nc) as tc:
        with (
            tc.tile_pool(name="weights", bufs=1) as wpool,
            tc.tile_pool(name="xin", bufs=1) as xpool,
            tc.tile_pool(name="hbuf", bufs=1) as hpool,
            tc.tile_pool(name="obuf", bufs=4) as opool,
            tc.tile_pool(name="ps1", bufs=3, space="PSUM") as psum1,
            tc.tile_pool(name="ps2", bufs=3, space="PSUM") as psum2,
            tc.tile_pool(name="warm", bufs=1, space="PSUM") as pswarm,
        ):
            # Warm up the PE clock with dummy matmuls while the input DMAs
            # stream in: 9x512-free cover the bulk (~3.8us cold), then
            # 24x128-free bridge at ~0.1us granularity to just before the
            # first real chain's data lands, so the PE never idles (an idle
            # gap would re-throttle the clock to 1.2GHz for ~3.4us).
            wz = xpool.tile([P, 512], bf16, tag="warmsrc")
            nc.gpsimd.memset(wz[:], 0.0)
            pw_t = pswarm.tile([P, 512], f32, tag="warmps")
            for _ in range(9):
                nc.tensor.matmul(pw_t[:], wz[:, :P], wz[:], start=True, stop=True)
            for _ in range(34):
                nc.tensor.matmul(pw_t[:, :P], wz[:, :P], wz[:, :P],
                                 start=True, stop=True)
            xt_sb = xpool.tile([P, KD * M], bf16)
            w1_sb = wpool.tile([P, KD * H], bf16)
            w2_sb = wpool.tile([P, DK * CH * P], bf16)
            CB = 4096  # 1 MiB blocks
            mo0, mb0 = m_blocks[0]
            # ramp-critical stream, all on the sync queue (q1): the first
            # stage-1 c-groups need xt block0 k=0..3 + W1 MiB0 first half;
            # their k=4..7 halves follow.
            h1 = KD // 2 * mb0
            nc.sync.dma_start(xt_sb[:, :h1], xt_d.ap()[:, :h1])
            nc.sync.dma_start(w1_sb[:, :CB // 2], w1_d.ap()[:, :CB // 2])
            nc.sync.dma_start(xt_sb[:, h1:KD * mb0], xt_d.ap()[:, h1:KD * mb0])
            nc.sync.dma_start(w1_sb[:, CB // 2:CB], w1_d.ap()[:, CB // 2:CB])
            for cb in range(1, KD * H // CB):
                nc.sync.dma_start(w1_sb[:, cb * CB:(cb + 1) * CB],
                                  w1_d.ap()[:, cb * CB:(cb + 1) * CB])
            # scalar queue: gelu table preload first, then biases (needed by
            # the first activation at ~16us)
            wg_t = xpool.tile([P, 1], bf16, tag="warmgelu")
            nc.scalar.activation(wg_t[:], wz[:, :1],
                                 mybir.ActivationFunctionType.Gelu)
            b1_sb = wpool.tile([P, CH], f32)
            nc.scalar.dma_start(b1_sb[:], b1_d.ap())
            # Bulk transfers not needed until ~40us+ are gated past the ramp:
            # concurrent early DMA measurably halves q1's throughput in the
            # critical first ~15us (total early-window DMA is capped), which
            # is exactly when the first chains' xt/W1 must land.
            wr_sb = wpool.tile([P, M], f32)
            deferred = [nc.scalar.dma_start(wr_sb[:], wr_d.ap())]
            if M > mb0:
                deferred.append(nc.scalar.dma_start(xt_sb[:, KD * mb0:],
                                                    xt_d.ap()[:, KD * mb0:]))
            for cb in range(DK * CH * P // CB):
                deferred.append(
                    nc.gpsimd.dma_start(w2_sb[:, cb * CB:(cb + 1) * CB],
                                        w2_d.ap()[:, cb * CB:(cb + 1) * CB]))

            def s1_mm(ps, c, k, fo, mb, start, stop):
                w1base = (c // 4) * 4096 + (c % 4) * P
                nc.tensor.matmul(
                    ps[:, :mb],
                    w1_sb[:, w1base + k * 512: w1base + k * 512 + P],
                    xt_sb[:, fo + k * mb: fo + (k + 1) * mb],
                    start=start,
                    stop=stop,
                )

            for bi, (mo, mb) in enumerate(m_blocks):
                fo = KD * mo
                # stage 1: hT[n, m] = gelu(sum_k W1[k, n] * x[m, k] + b1[n])
                ht = hpool.tile([P, CH, 512], bf16, tag="ht")
                c_start = 0
                if bi == 0:
                    # first three c-groups in two half-K passes: their k=0..3
                    # matmuls need only the first xt/W1 halves, so they run
                    # while the k=4..7 halves are still streaming in
                    c_start = 3
                    pss = []
                    for c in range(c_start):
                        ps = psum1.tile([P, 512], f32, tag="ps1")
                        pss.append(ps)
                        for k in range(KD // 2):
                            s1_mm(ps, c, k, fo, mb, k == 0, False)
                    for c in range(c_start):
                        ps = pss[c]
                        for k in range(KD // 2, KD):
                            s1_mm(ps, c, k, fo, mb, False, k == KD - 1)
                        act = nc.scalar.activation(
                            ht[:, c, :mb], ps[:, :mb],
                            mybir.ActivationFunctionType.Gelu,
                            bias=b1_sb[:, c:c + 1],
                        )
                        if c == 0 and deferred:
                            for dma in deferred:
                                tile.add_dep_helper(
                                    dma.ins, act.ins,
                                    reason="bulk DMA deferred past ramp")
                            deferred = []
                for c in range(c_start, CH):
                    ps = psum1.tile([P, 512], f32, tag="ps1")
                    for k in range(KD):
                        s1_mm(ps, c, k, fo, mb, k == 0, k == KD - 1)
                    nc.scalar.activation(
                        ht[:, c, :mb], ps[:, :mb],
                        mybir.ActivationFunctionType.Gelu,
                        bias=b1_sb[:, c:c + 1],
                    )
                # stage 2: yT[d, m] = w[m] * sum_h W2[h, d] * hT[h, m]
                # (d-chunk stationary, ht moving: cost ~ mb, no 128-tiling)
                last_block = (bi == len(m_blocks) - 1)
                for dk in range(DK):
                    last_dk = last_block and dk == DK - 1
                    ps2 = psum2.tile([P, 512], f32, tag="ps2")
                    for c in range(CH):
                        nc.tensor.matmul(
                            ps2[:, :mb],
                            w2_sb[:, dk * CH * P + c * P: dk * CH * P + (c + 1) * P],
                            ht[:, c, :mb],
                            start=(c == 0),
                            stop=(c == CH - 1),
                        )
                    ot = opool.tile([P, 512], f32, tag="ot")
                    if last_dk:
                        # final chain: evict in halves, store each on its own
                        # queue immediately so the last store overlaps the
                        # second evict instead of serializing after it
                        hw = mb // 2
                        for q, eng in ((0, nc.sync), (1, nc.scalar)):
                            nc.vector.tensor_mul(
                                ot[:, q * hw:(q + 1) * hw],
                                ps2[:, q * hw:(q + 1) * hw],
                                wr_sb[:, mo + q * hw: mo + (q + 1) * hw])
                            eng.dma_start(
                                y_d.ap()[dk * P:(dk + 1) * P,
                                         mo + q * hw: mo + (q + 1) * hw],
                                ot[:, q * hw:(q + 1) * hw])
                    else:
                        nc.vector.tensor_mul(ot[:, :mb], ps2[:, :mb],
                                             wr_sb[:, mo:mo + mb])
                        nc.sync.dma_start(
                            y_d.ap()[dk * P:(dk + 1) * P, mo:mo + mb],
                            ot[:, :mb])
    nc.compile()
    return nc


def _route(x2d, Wg, bg):
    """fp64 gating: returns (top2 indices [N,2], renormalized weights [N,2])."""
    logits = x2d.astype(np.float64) @ Wg.astype(np.float64) + bg.astype(np.float64)
    m = logits.max(-1, keepdims=True)
    e = np.exp(logits - m)
    gates = e / e.sum(-1, keepdims=True)
    top2 = np.argsort(-gates, axis=-1, kind="stable")[:, :K]
    g2 = np.take_along_axis(gates, top2, axis=-1)
    w2 = g2 / np.maximum(g2.sum(-1, keepdims=True), 1e-12)
    return top2, w2


def kernel(x, Wg, bg, W1, b1, W2, b2, _run_opts=None):
    from concourse.bass_utils import run_bass_kernel_spmd

    x = np.asarray(x)
    x2d = x.reshape(N_TOK, D)
    top2, wgt2 = _route(x2d, np.asarray(Wg), np.asarray(bg))

    # per-expert token lists
    pos = [np.where((top2 == e).any(-1))[0] for e in range(E)]
    pw = [
        (wgt2 * (top2 == e))[pos[e]].sum(-1).astype(np.float32)
        for e in range(E)
    ]
    max_n = max(len(p) for p in pos)
    M = max(P, -(-max_n // 16) * 16)

    if M not in _compiled_cache:
        _compiled_cache[M] = _build(M)
    nc = _compiled_cache[M]

    bf = ml_dtypes.bfloat16
    W1 = np.asarray(W1)
    W2 = np.asarray(W2)
    b1 = np.asarray(b1)
    b2 = np.asarray(b2)

    m_blocks = _m_blocks(M)

    in_maps = []
    for e in range(E):
        n_e = len(pos[e])
        xt = np.zeros((D, M), bf)
        xt[:, :n_e] = x2d[pos[e]].T.astype(bf)
        # pack xt -> [P, sum_b KD*mb] with per-block [k, m'] free layout
        xt3 = xt.reshape(KD, P, M)
        xtp = np.empty((P, KD * M), bf)
        for (mo, mb) in m_blocks:
            blk = xt3[:, :, mo:mo + mb]          # [KD, P, mb]
            xtp[:, KD * mo:KD * (mo + mb)] = (
                blk.transpose(1, 0, 2).reshape(P, KD * mb))
        # pack W1 [D, H] -> [P, KD*H]: free idx = cb*4096 + k*512 + h''
        w1p = (W1[e].astype(bf)
               .reshape(KD, P, H // 512, 512)      # [k, p, cb, h'']
               .transpose(1, 2, 0, 3)              # [p, cb, k, h'']
               .reshape(P, KD * H))
        # pack W2 [H, D] -> [P, DK*CH*P]: free idx = dk*CH*128 + c*128 + d'
        w2p = (W2[e].astype(bf)
               .reshape(CH, P, DK, P)              # [c, p, dk, d']
               .transpose(1, 2, 0, 3)              # [p, dk, c, d']
               .reshape(P, DK * CH * P))
        # gate weights replicated across partitions: [P, M]
        w_full = np.zeros((M,), np.float32)
        w_full[:n_e] = pw[e]
        wrep = np.broadcast_to(w_full, (P, M))
        in_maps.append({
            "xt": np.ascontiguousarray(xtp),
            "w1": np.ascontiguousarray(w1p),
            "w2": np.ascontiguousarray(w2p),
            "b1s": np.ascontiguousarray(b1[e].reshape(CH, P).T.astype(np.float32)),
            "wrep": np.ascontiguousarray(wrep),
        })

    try:
        res = run_bass_kernel_spmd(nc, in_maps, core_ids=list(range(E)),
                                   **(_run_opts or {}))
    except Exception:
        # transient device errors (e.g. NRT_EXEC_UNIT_UNRECOVERABLE) have
        # been observed on this fabric; one retry usually clears them
        res = run_bass_kernel_spmd(nc, in_maps, core_ids=list(range(E)),
                                   **(_run_opts or {}))

    out = np.zeros((N_TOK, D), np.float32)
    for e in range(E):
        n_e = len(pos[e])
        if n_e == 0:
            continue
        y = res.results[e]["y"][:, :n_e]          # [D, n_e]
        out[pos[e]] += y.T + pw[e][:, None] * b2[e][None, :].astype(np.float32)
    if _run_opts is not None:
        kernel._last_result = res
    return out.reshape(B, T, D)


if __name__ == "__main__":
    rng = np.random.default_rng(0)
    ins = {
        "x": rng.standard_normal((B, T, D), dtype=np.float32),
        "Wg": rng.standard_normal((D, E), dtype=np.float32) * 0.03,
        "bg": rng.standard_normal((E,), dtype=np.float32) * 0.03,
        "W1": rng.standard_normal((E, D, H), dtype=np.float32) * 0.03,
        "b1": rng.standard_normal((E, H), dtype=np.float32) * 0.03,
        "W2": rng.standard_normal((E, H, D), dtype=np.float32) * 0.015,
        "b2": rng.standard_normal((E, D), dtype=np.float32) * 0.015,
    }
    out = kernel(**ins)
    print("kernel out:", out.shape, out.dtype, float(np.abs(out).mean()))
